# revision 2
# baseline (speedup 1.0000x reference)
"""Trainium2 Bass kernel for the CMA momentum-memory update (nn_CMA_52956946760162).

Strategy (class-sharded, present-only compact packing, v2):
- Shard the C=4096 classes across 8 cores (512 classes/core), no collectives.
- Host packs, per (core, modality), the *present* (label,cam) segments and
  present labels into chunks of <=128 slots (whole classes per chunk), slots
  ordered [a!=0 | a==0 | pad] so the memory-bank read covers only the a!=0
  prefix. Per-chunk row/slot counts are compiled into the program (maxima
  across cores), so every DMA moves exactly the bytes that are used.
- The one-hot matrix is built on-device from 4 packed f32 coefficients per
  feature row (seg column, seg coeff, class column, class coeff) via
  iota==col compares, so the feature stream carries 16B/row of metadata
  instead of 512B.  One tensor-engine pass per chunk produces the scaled
  per-(label,cam) and per-label sums in PSUM; a fused DVE op blends with the
  momentum-scaled memory rows (a==0 rows pass the raw mean through).
- Rows absent from the batch leave memory unchanged; the host passes them
  through directly from the input banks during output assembly and scatters
  the device-computed rows over them.
"""

import numpy as np

C, K, D, N = 4096, 6, 2048, 16384
SIGMA = 0.2
M = 8                 # cores
CPC = C // M          # classes per core = 512
CK = C * K
F32 = np.float32

_BUILD_CACHE = {}


def _pack_core_modality(core, feats, labels, cams, valid):
    """Pack one (core, modality) into chunks of whole classes.

    Returns a list of chunk dicts with:
      n1, u, R           : a!=0 slot count, used slot count, feature rows
      frows [R]          : global feat row indices (class-grouped order)
      colc/bc/colg/bg [R]: per-feature-row one-hot coefficients
      mem_src [n1]       : merged bank row ids (class c -> c, seg s -> CPC+s)
      out_tgt [u]        : merged bank row ids for output scatter
    """
    c0 = core * CPC
    mask = (labels >= c0) & (labels < c0 + CPC)
    rows_all = np.nonzero(mask)[0]
    lab = labels[rows_all] - c0
    seg = lab * K + cams[rows_all]
    order = np.argsort(seg, kind="stable")
    rows_all, lab, seg = rows_all[order], lab[order], seg[order]

    ccnt = np.bincount(seg, minlength=CPC * K)
    gcnt = np.bincount(lab, minlength=CPC)
    v = np.asarray(valid[c0:c0 + CPC]).reshape(CPC * K)
    cpres = ccnt > 0
    class_start = np.searchsorted(lab, np.arange(CPC + 1))

    vp = (cpres & v).reshape(CPC, K)       # valid-present segs per class
    ip = (cpres & ~v).reshape(CPC, K)      # invalid-present segs per class
    n1_of = vp.sum(axis=1) + 1             # +1 for the class column
    n0_of = ip.sum(axis=1)
    present = np.nonzero(gcnt > 0)[0]

    # greedy whole-class packing: rows <= 128, total slots <= 128
    chunk_classes = []
    cur, rws, cols = [], 0, 0
    for c in present:
        nr = int(gcnt[c])
        ncol = int(n1_of[c] + n0_of[c])
        if cur and (rws + nr > 128 or cols + ncol > 128):
            chunk_classes.append(cur)
            cur, rws, cols = [], 0, 0
        cur.append(int(c))
        rws += nr
        cols += ncol
    if cur:
        chunk_classes.append(cur)

    b_c_all = np.where(v, SIGMA, 1.0) / np.maximum(ccnt, 1)     # per seg
    b_g_all = SIGMA / np.maximum(gcnt, 1)                       # per class

    chunks = []
    for cls_list in chunk_classes:
        slot_of_seg = {}
        slot_of_cls = {}
        mem_src, out_tgt = [], []
        for c in cls_list:                      # a != 0 slots first
            for kk in np.nonzero(vp[c])[0]:
                s = c * K + int(kk)
                slot_of_seg[s] = len(out_tgt)
                mem_src.append(CPC + s)
                out_tgt.append(CPC + s)
            slot_of_cls[c] = len(out_tgt)
            mem_src.append(c)
            out_tgt.append(c)
        n1 = len(out_tgt)
        for c in cls_list:                      # a == 0 slots (raw mean)
            for kk in np.nonzero(ip[c])[0]:
                s = c * K + int(kk)
                slot_of_seg[s] = len(out_tgt)
                out_tgt.append(CPC + s)
        u = len(out_tgt)

        fr, colc, bc, colg, bg = [], [], [], [], []
        for c in cls_list:
            r0, r1 = int(class_start[c]), int(class_start[c + 1])
            for r in range(r0, r1):
                fr.append(rows_all[r])
                s = int(seg[r])
                colc.append(slot_of_seg[s])
                bc.append(b_c_all[s])
                colg.append(slot_of_cls[c])
                bg.append(b_g_all[c])
        chunks.append(dict(
            n1=n1, u=u, R=len(fr),
            frows=np.asarray(fr, np.int64),
            colc=np.asarray(colc, F32), bc=np.asarray(bc, F32),
            colg=np.asarray(colg, F32), bg=np.asarray(bg, F32),
            mem_src=np.asarray(mem_src, np.int64),
            out_tgt=np.asarray(out_tgt, np.int64)))
    return chunks


def _build_program(prof):
    """Build + compile the SPMD Bass program for a chunk profile.

    prof: tuple of (R, N1, U) per chunk slot (both modalities concatenated).
    """
    import concourse.mybir as mybir
    import concourse.tile as tile
    from concourse import bacc

    f32 = mybir.dt.float32
    eq = mybir.AluOpType.is_equal
    mult = mybir.AluOpType.mult
    add = mybir.AluOpType.add
    nc = bacc.Bacc("TRN2", target_bir_lowering=False, debug=False)

    NT = len(prof)
    TR = sum(p[0] for p in prof)
    TM = sum(p[1] for p in prof)
    TO = sum(p[2] for p in prof)
    MEM_BUFS = 4
    fpoh = nc.dram_tensor("fpoh", [TR, D + 4], f32, kind="ExternalInput").ap()
    memin = nc.dram_tensor("memin", [max(TM, 1), D], f32, kind="ExternalInput").ap()
    avec = nc.dram_tensor("avec", [128, NT], f32, kind="ExternalInput").ap()
    iota = nc.dram_tensor("iota", [128, 128], f32, kind="ExternalInput").ap()
    out = nc.dram_tensor("out", [TO, D], f32, kind="ExternalOutput").ap()

    with tile.TileContext(nc) as tc:
        with tc.tile_pool(name="const", bufs=1) as constp, \
             tc.tile_pool(name="io", bufs=4) as iop, \
             tc.tile_pool(name="ps", bufs=2, space="PSUM") as psp:

            avec_t = constp.tile([128, NT], f32, name="avec_t")
            nc.sync.dma_start(out=avec_t[:], in_=avec[:, :])
            iota_t = constp.tile([128, 128], f32, name="iota_t")
            nc.sync.dma_start(out=iota_t[:], in_=iota[:, :])

            # pre-zero the mem tiles so a==0 slots never see NaN garbage
            for _ in range(MEM_BUFS):
                mem_sb = iop.tile([128, D], f32, tag="mem", bufs=MEM_BUFS,
                                  name="mem_sb")
                nc.gpsimd.memset(mem_sb[:], 0.0)

            fo = mo = oo = 0
            for j, (R, N1, U) in enumerate(prof):
                frow = iop.tile([128, D + 4], f32, tag="frow", bufs=4,
                                name="frow")
                nc.sync.dma_start(out=frow[0:R, :], in_=fpoh[fo:fo + R, :])
                ohc = iop.tile([128, 128], f32, tag="ohc", bufs=3, name="ohc")
                oh = iop.tile([128, 128], f32, tag="oh", bufs=3, name="oh")
                nc.vector.tensor_scalar(
                    out=ohc[0:R, :], in0=iota_t[0:R, :],
                    scalar1=frow[0:R, D:D + 1], scalar2=frow[0:R, D + 1:D + 2],
                    op0=eq, op1=mult)
                nc.vector.tensor_scalar(
                    out=oh[0:R, :], in0=iota_t[0:R, :],
                    scalar1=frow[0:R, D + 2:D + 3], scalar2=frow[0:R, D + 3:D + 4],
                    op0=eq, op1=mult)
                nc.vector.scalar_tensor_tensor(
                    out=oh[0:R, :], in0=ohc[0:R, :], scalar=1.0,
                    in1=oh[0:R, :], op0=mult, op1=add)

                psum = psp.tile([128, D], f32, tag="ps", name="psum")
                for t in range(4):
                    sl = slice(t * 512, (t + 1) * 512)
                    nc.tensor.matmul(psum[:, sl], oh[0:R, :], frow[0:R, sl],
                                     start=True, stop=True)

                mem_sb = iop.tile([128, D], f32, tag="mem", bufs=MEM_BUFS,
                                  name="mem_sb")
                nc.scalar.dma_start(out=mem_sb[0:N1, :],
                                    in_=memin[mo:mo + N1, :])
                out_sb = iop.tile([128, D], f32, tag="out", bufs=6,
                                  name="out_sb")
                nc.vector.scalar_tensor_tensor(
                    out=out_sb[0:U, :], in0=mem_sb[0:U, :],
                    scalar=avec_t[0:U, j:j + 1], in1=psum[0:U, :],
                    op0=mult, op1=add)
                nc.gpsimd.dma_start(out=out[oo:oo + U, :], in_=out_sb[0:U, :])
                fo, mo, oo = fo + R, mo + N1, oo + U

    nc.compile()
    return nc


def prepare(inputs):
    """Build (or reuse) the program and the per-core input maps + scatter maps."""
    a = {k: np.ascontiguousarray(np.asarray(v)) for k, v in inputs.items()}
    mods = [
        (a["rgb_feats"], a["rgb_labels"].astype(np.int64), a["rgb_cams"].astype(np.int64),
         a["vis_cam_valid"], a["vis_memory"], a["vis_cam_memory"].reshape(CK, D)),
        (a["ir_feats"], a["ir_labels"].astype(np.int64), a["ir_cams"].astype(np.int64),
         a["ir_cam_valid"], a["ir_memory"], a["ir_cam_memory"].reshape(CK, D)),
    ]

    # pack every (core, modality); chunk slot layout = mod0 chunks ++ mod1 chunks
    packs = [[_pack_core_modality(core, f, l, cm, v)
              for (f, l, cm, v, _, _) in mods] for core in range(M)]
    nch = [max(len(packs[core][m]) for core in range(M)) for m in range(2)]
    prof = []
    for m in range(2):
        for j in range(nch[m]):
            R = N1 = U = 0
            for core in range(M):
                ch = packs[core][m]
                if j < len(ch):
                    R = max(R, ch[j]["R"])
                    N1 = max(N1, ch[j]["n1"])
                    U = max(U, ch[j]["u"])
            prof.append((R, N1, U))
    prof = tuple(prof)

    if prof not in _BUILD_CACHE:
        _BUILD_CACHE.clear()
        _BUILD_CACHE[prof] = _build_program(prof)
    nc = _BUILD_CACHE[prof]

    NT = len(prof)
    TR = sum(p[0] for p in prof)
    TM = max(sum(p[1] for p in prof), 1)
    TO = sum(p[2] for p in prof)
    iota_np = np.broadcast_to(np.arange(128, dtype=F32), (128, 128)).copy()

    in_maps, metas = [], []
    for core in range(M):
        c0 = core * CPC
        fpoh = np.zeros((TR, D + 4), F32)
        memin = np.zeros((TM, D), F32)
        avec = np.zeros((128, NT), F32)
        meta = []
        slot = 0
        fo = mo = oo = 0
        for m in range(2):
            feats, _, _, _, gmem, cmem = mods[m]
            obase = (C + CK) * m
            ch_list = packs[core][m]
            for j in range(nch[m]):
                R, N1, U = prof[slot]
                if j < len(ch_list):
                    ch = ch_list[j]
                    r = ch["R"]
                    fpoh[fo:fo + r, :D] = feats[ch["frows"]]
                    fpoh[fo:fo + r, D] = ch["colc"]
                    fpoh[fo:fo + r, D + 1] = ch["bc"]
                    fpoh[fo:fo + r, D + 2] = ch["colg"]
                    fpoh[fo:fo + r, D + 3] = ch["bg"]
                    src = ch["mem_src"]
                    isg = src < CPC
                    memin[mo:mo + ch["n1"]][isg] = gmem[c0 + src[isg]]
                    memin[mo:mo + ch["n1"]][~isg] = cmem[core * CPC * K + (src[~isg] - CPC)]
                    avec[0:ch["n1"], slot] = 1.0 - SIGMA
                    tgt = ch["out_tgt"]
                    oisg = tgt < CPC
                    gl = np.where(oisg, obase + c0 + tgt,
                                  obase + C + core * CPC * K + (tgt - CPC))
                    meta.append((oo, ch["u"], gl))
                fo, mo, oo = fo + R, mo + N1, oo + U
                slot += 1
        in_maps.append({"fpoh": fpoh, "memin": memin, "avec": avec,
                        "iota": iota_np})
        metas.append(meta)
    return nc, in_maps, metas, a, mods


def assemble(a, mods, metas, results):
    full = np.concatenate([a["vis_memory"], mods[0][5], a["ir_memory"], mods[1][5]],
                          axis=0).astype(F32, copy=True)
    for core in range(M):
        o = results[core]["out"]
        for oo, u, gl in metas[core]:
            full[gl] = o[oo:oo + u]
    return full


def kernel(**inputs):
    from concourse.bass_utils import run_bass_kernel_spmd

    nc, in_maps, metas, a, mods = prepare(inputs)
    res = run_bass_kernel_spmd(nc, in_maps, core_ids=list(range(M)))
    return assemble(a, mods, metas, res.results)


# revision 9
# speedup vs baseline: 2.1779x; 2.1779x over previous
"""Trainium2 Bass kernel for the CMA momentum-memory update (nn_CMA_52956946760162).

Strategy (class-sharded, full-tile packing with rotating mem stream, v3):
- Shard the C=4096 classes across 8 cores (512 classes/core), no collectives.
- Both modalities are packed into ONE chunk stream per core.  Every chunk is
  exactly 128 feature rows and <=128 psum slots; classes/segments straddling a
  chunk boundary are split, with secondary partial-sum slots merged by a tiny
  host-side add during assembly.  All device DMAs are full-128-partition
  (partial-partition DMAs starve the 16 SDMA engines in a mixed stream).
- The memory-bank rows needed for the momentum blend (valid-present segments
  and present classes, i.e. rows with blend coefficient a=0.8) form a single
  dense stream, DMAed as full [128, D] tiles decoupled from chunks.  Each
  such slot's psum partition equals its stream position mod 128, so the blend
  is two partition-ranged DVE scalar_tensor_tensor ops per chunk (split at
  the tile boundary).  a=0 slots (invalid-present -> raw mean) multiply
  whatever the mem tile holds by 0, so they may sit on any partition.
- The one-hot matrix is built on-device from 4 packed f32 coefficients per
  feature row (seg column, seg coeff, class column, class coeff) via
  iota==col compares; the feature stream carries 16B/row of metadata.
- Rows absent from the batch leave memory unchanged; the host passes them
  through from the input banks during output assembly and scatters the
  device-computed rows over them.
"""

import numpy as np

C, K, D, N = 4096, 6, 2048, 16384
SIGMA = 0.2
M = 8                 # cores
CPC = C // M          # classes per core = 512
CK = C * K
F32 = np.float32

_BUILD_CACHE = {}


class _Chunk:
    __slots__ = ("rows", "colc", "bc", "colg", "bg", "n1", "a1_src",
                 "a1_tgt", "a0_tgt", "a0_prim")

    def __init__(self):
        self.rows = []      # global feat-row ids (with modality offset)
        self.colc = []      # per row: slot ref of its seg column
        self.bc = []        # per row: seg coefficient
        self.colg = []      # per row: slot ref of its class column
        self.bg = []        # per row: class coefficient
        self.n1 = 0         # number of a!=0 slots (mem stream positions)
        self.a1_src = []    # per a!=0 slot: merged bank row id
        self.a1_tgt = []    # per a!=0 slot: merged out row id
        self.a0_tgt = []    # per a=0 slot: merged out row id
        self.a0_prim = []   # per a=0 slot: True if primary (assign), else add


def _pack_core(core, mods):
    """Pack one core's work (both modalities) into exact-128-row chunks.

    Bank row ids are encoded per modality m as base + row, where
    base = m * (CPC + CPC*K); class c -> base + c, seg s -> base + CPC + s.
    Feature row ids are encoded as m * N + row.
    Returns (chunks, targets are resolved later by the caller).
    """
    c0 = core * CPC
    chunks = [_Chunk()]
    cur = chunks[-1]
    # stream position bookkeeping happens later (per-chunk n1 + profile)

    def close():
        nonlocal cur
        chunks.append(_Chunk())
        cur = chunks[-1]

    for m, (feats, labels, cams, valid, gmem, cmem) in enumerate(mods):
        base = m * (CPC + CPC * K)
        mask = (labels >= c0) & (labels < c0 + CPC)
        rows_all = np.nonzero(mask)[0]
        lab = labels[rows_all] - c0
        seg = lab * K + cams[rows_all]
        order = np.argsort(seg, kind="stable")
        rows_all, lab, seg = rows_all[order], lab[order], seg[order]
        ccnt = np.bincount(seg, minlength=CPC * K)
        gcnt = np.bincount(lab, minlength=CPC)
        v = np.asarray(valid[c0:c0 + CPC]).reshape(CPC * K)
        class_start = np.searchsorted(lab, np.arange(CPC + 1))
        b_c_all = np.where(v, SIGMA, 1.0) / np.maximum(ccnt, 1)
        b_g_all = SIGMA / np.maximum(gcnt, 1)

        for c in np.nonzero(gcnt > 0)[0]:
            c = int(c)
            r0, r1 = int(class_start[c]), int(class_start[c + 1])
            # segments of this class: (seg_id, row_ids)
            segs = []
            i = r0
            while i < r1:
                jn = i
                while jn < r1 and seg[jn] == seg[i]:
                    jn += 1
                segs.append((int(seg[i]), rows_all[i:jn]))
                i = jn
            bg = float(b_g_all[c])
            class_primary_placed = False
            si, srow = 0, 0     # next seg index / next row within it
            while si < len(segs):
                # room check: need >= 2 slots (class col + 1 seg) and >= 1 row
                if len(cur.rows) >= 128 or \
                   cur.n1 + len(cur.a0_tgt) >= 127:
                    close()
                # place class column for this chunk
                if class_primary_placed:
                    cur.a0_tgt.append(base + c)
                    cur.a0_prim.append(False)
                    gslot_ref = ("a0", len(cur.a0_tgt) - 1)
                else:
                    cur.a1_src.append(base + c)
                    cur.a1_tgt.append(base + c)
                    gslot_ref = ("a1", cur.n1)
                    cur.n1 += 1
                    class_primary_placed = True
                placed_rows = False
                while si < len(segs):
                    s, srows = segs[si]
                    if cur.n1 + len(cur.a0_tgt) >= 128:
                        break
                    room = 128 - len(cur.rows)
                    if room == 0:
                        break
                    take = min(len(srows) - srow, room)
                    seg_primary = (srow == 0)
                    bc = float(b_c_all[s])
                    if seg_primary and v[s]:
                        cur.a1_src.append(base + CPC + s)
                        cur.a1_tgt.append(base + CPC + s)
                        sslot_ref = ("a1", cur.n1)
                        cur.n1 += 1
                    else:
                        cur.a0_tgt.append(base + CPC + s)
                        cur.a0_prim.append(seg_primary)
                        sslot_ref = ("a0", len(cur.a0_tgt) - 1)
                    for r in srows[srow:srow + take]:
                        cur.rows.append(m * N + int(r))
                        cur.colc.append(sslot_ref)
                        cur.bc.append(bc)
                        cur.colg.append(gslot_ref)
                        cur.bg.append(bg)
                    placed_rows = True
                    srow += take
                    if srow == len(srows):
                        si += 1
                        srow = 0
                    else:
                        break   # chunk rows full; seg continues next chunk
                if not placed_rows:
                    # undo the class column we just placed in this chunk
                    if gslot_ref[0] == "a0":
                        cur.a0_tgt.pop()
                        cur.a0_prim.pop()
                    else:
                        cur.a1_src.pop()
                        cur.a1_tgt.pop()
                        cur.n1 -= 1
                        class_primary_placed = False
                    close()
    if not chunks[-1].rows:
        chunks.pop()
    return chunks


def _build_program(prof):
    """Build + compile the SPMD Bass program.

    prof: tuple of N1_j per chunk (rows=128, slots<=128, out full tile).
    """
    import concourse.mybir as mybir
    import concourse.tile as tile
    from concourse import bacc

    f32 = mybir.dt.float32
    eq = mybir.AluOpType.is_equal
    mult = mybir.AluOpType.mult
    add = mybir.AluOpType.add
    nc = bacc.Bacc("TRN2", target_bir_lowering=False, debug=False)

    NT = len(prof)
    TM = sum(prof)
    TMT = (TM + 127) // 128          # number of full mem tiles
    MEM_BUFS = 5
    fpoh = nc.dram_tensor("fpoh", [NT * 128, D + 4], f32, kind="ExternalInput").ap()
    memin = nc.dram_tensor("memin", [TMT * 128, D], f32, kind="ExternalInput").ap()
    avec = nc.dram_tensor("avec", [128, NT], f32, kind="ExternalInput").ap()
    iota = nc.dram_tensor("iota", [128, 128], f32, kind="ExternalInput").ap()
    out = nc.dram_tensor("out", [NT * 128, D], f32, kind="ExternalOutput").ap()

    with tile.TileContext(nc) as tc:
        with tc.tile_pool(name="const", bufs=1) as constp, \
             tc.tile_pool(name="io", bufs=4) as iop, \
             tc.tile_pool(name="ps", bufs=2, space="PSUM") as psp:

            avec_t = constp.tile([128, NT], f32, name="avec_t")
            nc.sync.dma_start(out=avec_t[:], in_=avec[:, :])
            iota_t = constp.tile([128, 128], f32, name="iota_t")
            nc.sync.dma_start(out=iota_t[:], in_=iota[:, :])

            mem_tiles = {}
            loaded = 0

            def load_mem_upto(t):
                nonlocal loaded
                while loaded <= t and loaded < TMT:
                    mt = iop.tile([128, D], f32, tag="mem", bufs=MEM_BUFS,
                                  name="mem_sb")
                    nc.scalar.dma_start(
                        out=mt[:],
                        in_=memin[loaded * 128:(loaded + 1) * 128, :])
                    mem_tiles[loaded] = mt
                    loaded += 1

            S = 0
            for j, N1 in enumerate(prof):
                p = S % 128
                t = S // 128
                load_mem_upto(min(t + 1, TMT - 1))

                frow = iop.tile([128, D + 4], f32, tag="frow", bufs=4,
                                name="frow")
                nc.sync.dma_start(out=frow[:], in_=fpoh[j * 128:(j + 1) * 128, :])
                ohc = iop.tile([128, 128], f32, tag="ohc", bufs=3, name="ohc")
                oh = iop.tile([128, 128], f32, tag="oh", bufs=3, name="oh")
                nc.vector.tensor_scalar(
                    out=ohc[:], in0=iota_t[:],
                    scalar1=frow[:, D:D + 1], scalar2=frow[:, D + 1:D + 2],
                    op0=eq, op1=mult)
                nc.vector.tensor_scalar(
                    out=oh[:], in0=iota_t[:],
                    scalar1=frow[:, D + 2:D + 3], scalar2=frow[:, D + 3:D + 4],
                    op0=eq, op1=mult)
                nc.vector.scalar_tensor_tensor(
                    out=oh[:], in0=ohc[:], scalar=1.0, in1=oh[:],
                    op0=mult, op1=add)

                psum = psp.tile([128, D], f32, tag="ps", name="psum")
                for tt in range(4):
                    sl = slice(tt * 512, (tt + 1) * 512)
                    nc.tensor.matmul(psum[:, sl], oh[:], frow[:, sl],
                                     start=True, stop=True)

                mem_a = mem_tiles[t]
                mem_b = mem_tiles.get(t + 1, mem_a)
                out_sb = iop.tile([128, D], f32, tag="out", bufs=6,
                                  name="out_sb")
                # ops with a non-zero partition start are limited to 32
                # partitions, so emit the [p:128) piece in 32-row strips
                for q in (range(p, 128, 32) if p else [0]):
                    qe = q + 32 if p else 128
                    nc.vector.scalar_tensor_tensor(
                        out=out_sb[q:qe, :], in0=mem_a[q:qe, :],
                        scalar=avec_t[q:qe, j:j + 1], in1=psum[q:qe, :],
                        op0=mult, op1=add)
                if p > 0:
                    nc.vector.scalar_tensor_tensor(
                        out=out_sb[0:p, :], in0=mem_b[0:p, :],
                        scalar=avec_t[0:p, j:j + 1], in1=psum[0:p, :],
                        op0=mult, op1=add)
                nc.gpsimd.dma_start(out=out[j * 128:(j + 1) * 128, :],
                                    in_=out_sb[:])
                # free tiles no longer needed
                S += N1
                tnext = S // 128
                for told in [k for k in mem_tiles if k < tnext]:
                    del mem_tiles[told]

    nc.compile()
    return nc


def prepare(inputs):
    """Pack, build (or reuse) the program, and build per-core input maps."""
    a = {k: np.ascontiguousarray(np.asarray(v)) for k, v in inputs.items()}
    mods = [
        (a["rgb_feats"], a["rgb_labels"].astype(np.int64), a["rgb_cams"].astype(np.int64),
         a["vis_cam_valid"], a["vis_memory"], a["vis_cam_memory"].reshape(CK, D)),
        (a["ir_feats"], a["ir_labels"].astype(np.int64), a["ir_cams"].astype(np.int64),
         a["ir_cam_valid"], a["ir_memory"], a["ir_cam_memory"].reshape(CK, D)),
    ]
    packs = [_pack_core(core, mods) for core in range(M)]
    nch = max(len(p) for p in packs)
    # round stream consumption to multiples of 32 so the rotation offset p
    # stays 32-aligned (DVE partition ranges must start 32-aligned)
    prof = tuple((max(packs[core][j].n1 if j < len(packs[core]) else 0
                      for core in range(M)) + 31) // 32 * 32
                 for j in range(nch))

    if prof not in _BUILD_CACHE:
        _BUILD_CACHE.clear()
        _BUILD_CACHE[prof] = _build_program(prof)
    nc = _BUILD_CACHE[prof]

    NT = len(prof)
    TM = sum(prof)
    TMT = (TM + 127) // 128
    iota_np = np.broadcast_to(np.arange(128, dtype=F32), (128, 128)).copy()
    feats_all = [mods[0][0], mods[1][0]]

    in_maps, metas = [], []
    for core in range(M):
        chunks = packs[core]
        fpoh = np.zeros((NT * 128, D + 4), F32)
        memin = np.zeros((TMT * 128, D), F32)
        avec = np.zeros((128, NT), F32)
        prim_src, prim_tgt = [], []   # out-buffer row -> global out row (assign)
        sec_src, sec_tgt = [], []     # secondary pieces (added)
        S = 0
        for j in range(NT):
            p = S % 128
            N1 = prof[j]
            if j >= len(chunks):
                S += N1
                continue
            ch = chunks[j]
            n1 = ch.n1
            a1_part = [(p + i) % 128 for i in range(n1)]
            used = set(a1_part)
            free = [q for q in range(128) if q not in used]
            assert len(ch.a0_tgt) <= len(free)
            a0_part = free[:len(ch.a0_tgt)]
            avec[a1_part, j] = 1.0 - SIGMA

            def part_of(ref):
                kind, idx = ref
                return a1_part[idx] if kind == "a1" else a0_part[idx]

            r0 = j * 128
            nr = len(ch.rows)
            rows = np.asarray(ch.rows)
            mrow = rows // N
            frow = rows % N
            for m in (0, 1):
                sel = mrow == m
                if sel.any():
                    fpoh[r0:r0 + nr, :D][sel] = feats_all[m][frow[sel]]
            fpoh[r0:r0 + nr, D] = [part_of(x) for x in ch.colc]
            fpoh[r0:r0 + nr, D + 1] = ch.bc
            fpoh[r0:r0 + nr, D + 2] = [part_of(x) for x in ch.colg]
            fpoh[r0:r0 + nr, D + 3] = ch.bg

            # mem stream rows for this chunk: positions S .. S+n1
            for i, src in enumerate(ch.a1_src):
                memin[S + i] = _bank_row(src, core, mods)
            # out targets
            for i, tgt in enumerate(ch.a1_tgt):
                prim_src.append(r0 + a1_part[i])
                prim_tgt.append(_out_row(tgt, core))
            for i, tgt in enumerate(ch.a0_tgt):
                (prim_src if ch.a0_prim[i] else sec_src).append(r0 + a0_part[i])
                (prim_tgt if ch.a0_prim[i] else sec_tgt).append(_out_row(tgt, core))
            S += N1
        in_maps.append({"fpoh": fpoh, "memin": memin, "avec": avec,
                        "iota": iota_np})
        metas.append((np.asarray(prim_src, np.int64), np.asarray(prim_tgt, np.int64),
                      np.asarray(sec_src, np.int64), np.asarray(sec_tgt, np.int64)))
    return nc, in_maps, metas, a, mods


_BANK_BASE = CPC + CPC * K


def _bank_row(src, core, mods):
    m, r = divmod(src, _BANK_BASE)
    if r < CPC:
        return mods[m][4][core * CPC + r]
    return mods[m][5][core * CPC * K + (r - CPC)]


def _out_row(tgt, core):
    m, r = divmod(tgt, _BANK_BASE)
    obase = (C + CK) * m
    if r < CPC:
        return obase + core * CPC + r
    return obase + C + core * CPC * K + (r - CPC)


def assemble(a, mods, metas, results):
    full = np.concatenate([a["vis_memory"], mods[0][5], a["ir_memory"], mods[1][5]],
                          axis=0).astype(F32, copy=True)
    for core in range(M):
        o = results[core]["out"]
        prim_src, prim_tgt, sec_src, sec_tgt = metas[core]
        full[prim_tgt] = o[prim_src]
        if len(sec_src):
            np.add.at(full, sec_tgt, o[sec_src])
    return full


def kernel(**inputs):
    from concourse.bass_utils import run_bass_kernel_spmd

    nc, in_maps, metas, a, mods = prepare(inputs)
    res = run_bass_kernel_spmd(nc, in_maps, core_ids=list(range(M)))
    return assemble(a, mods, metas, res.results)
